# revision 1
# baseline (speedup 1.0000x reference)
"""Trainium2 Bass kernel for nn_MultiLayerPerceptron_he_36412732735948.

GCN + MLP on B=32 point clouds of N=1024 nodes. Pure data parallel:
batch sharded 4-per-core across 8 NeuronCores, weights replicated.

Key algebraic restructurings (validated in numpy to rel-err ~1e-6):
  * dist^2 via matmul: d2[i,j] = r2_i + r2_j - 2(x_i x_j + y_i y_j),
    computed exactly-enough with a 3-part bf16 split. The 6 split-pair
    groups are materialized as 24 K-rows of two [128, N] operand
    tensors (per-batch 32-row blocks) so one matmul streams each tile.
  * adjacency kept as the SIGN matrix s = sign(T - d2) in {-1,+1}
    (exact in bf16); deg comes free from the ACT accum_out of the same
    instruction; adj = (s+1)/2 algebra pushed into the matmuls.
  * mean_i(nadj @ h) collapses: sum_i dinv_i adj_ij dinv_j h_jk =
    sum_j (w dinv)_j h_jk with w = adj @ dinv, so the second GCN layer
    is a matvec, not an [N,N]@[N,256] matmul.
  * u_j > 0 strictly, so u_j relu(q_jk) = relu(u_j q_jk): the weighted
    node-sum of relu(h W1) folds into relu + free-dim accumulation.
  * max(dist^2) reduced over the upper triangle only (symmetry).
"""

import sys

if "/opt/trn_rl_repo" not in sys.path:
    sys.path.insert(0, "/opt/trn_rl_repo")

import numpy as np

import concourse.bacc as bacc
import concourse.bass as bass
import concourse.bass_isa as bass_isa
import concourse.tile as tile
from concourse import masks, mybir
from concourse.bass_utils import run_bass_kernel_spmd

F32 = mybir.dt.float32
BF16 = mybir.dt.bfloat16
AF = mybir.ActivationFunctionType
ALU = mybir.AluOpType

B, N, FEAT = 32, 1024, 7
NCORES = 8
BL = B // NCORES          # batches per core
NT = N // 128             # node tiles
HID = 256
MLP_H = 64
OUT = 8
KFLAT = N * FEAT          # 7168
NKT = KFLAT // 128        # 56

# d2 split-pair groups: (L-part, R-part) per 4-row block
LA = [0, 0, 1, 1, 0, 2]
RA = [0, 1, 0, 1, 2, 0]


def _threshold() -> float:
    """Smallest fp32 d2 with sqrt_f32(d2) >= 0.3f; then (d2 < T) == (sqrt(d2) < 0.3f)."""
    f3 = np.float32(0.3)
    c = np.float32(f3 * f3)
    for _ in range(200):
        if np.sqrt(c) >= f3:
            c = np.nextafter(c, np.float32(0), dtype=np.float32)
        else:
            break
    while np.sqrt(np.nextafter(c, np.float32(1), dtype=np.float32)) < f3:
        c = np.nextafter(c, np.float32(1), dtype=np.float32)
    return float(np.nextafter(c, np.float32(1), dtype=np.float32))


THRESH = _threshold()

_NC_CACHE = {}


def _build():
    nc = bacc.Bacc("TRN2", target_bir_lowering=False, debug=False)

    x_d = nc.dram_tensor("x", (BL, N, FEAT), F32, kind="ExternalInput")
    w1_d = nc.dram_tensor("W1", (HID, 2), F32, kind="ExternalInput")
    b1_d = nc.dram_tensor("b1", (HID,), F32, kind="ExternalInput")
    w2_d = nc.dram_tensor("W2", (HID, HID), F32, kind="ExternalInput")
    b2_d = nc.dram_tensor("b2", (HID,), F32, kind="ExternalInput")
    wfc_d = nc.dram_tensor("Wfc", (HID, HID), F32, kind="ExternalInput")
    bfc_d = nc.dram_tensor("bfc", (HID,), F32, kind="ExternalInput")
    wg_d = nc.dram_tensor("Wg", (8, 2), F32, kind="ExternalInput")
    bg_d = nc.dram_tensor("bg", (8,), F32, kind="ExternalInput")
    wm0_d = nc.dram_tensor("Wm0", (MLP_H, KFLAT), F32, kind="ExternalInput")
    bm0_d = nc.dram_tensor("bm0", (MLP_H,), F32, kind="ExternalInput")
    wm1_d = nc.dram_tensor("Wm1", (MLP_H, MLP_H), F32, kind="ExternalInput")
    bm1_d = nc.dram_tensor("bm1", (MLP_H,), F32, kind="ExternalInput")
    wp_d = nc.dram_tensor("Wp", (OUT, MLP_H + HID + 8), F32, kind="ExternalInput")
    bp_d = nc.dram_tensor("bp", (OUT,), F32, kind="ExternalInput")
    out_d = nc.dram_tensor("out", (BL, OUT), F32, kind="ExternalOutput")

    with tile.TileContext(nc) as tc:
        _emit(nc, tc, x_d, w1_d, b1_d, w2_d, b2_d, wfc_d, bfc_d, wg_d, bg_d,
              wm0_d, bm0_d, wm1_d, bm1_d, wp_d, bp_d, out_d)
    nc.compile()
    return nc


def _emit(nc, tc, x_d, w1_d, b1_d, w2_d, b2_d, wfc_d, bfc_d, wg_d, bg_d,
          wm0_d, bm0_d, wm1_d, bm1_d, wp_d, bp_d, out_d):
    from contextlib import ExitStack
    ctx = ExitStack()

    const = ctx.enter_context(tc.tile_pool(name="const", bufs=1))
    work = ctx.enter_context(tc.tile_pool(name="work", bufs=2))
    spool = ctx.enter_context(tc.tile_pool(name="spool", bufs=2))
    scratch = ctx.enter_context(tc.tile_pool(name="scratch", bufs=2))

    d2pool = ctx.enter_context(tc.tile_pool(name="d2ps", bufs=2, space="PSUM"))
    szpool = ctx.enter_context(tc.tile_pool(name="szps", bufs=1, space="PSUM"))
    smpool = ctx.enter_context(tc.tile_pool(name="smps", bufs=1, space="PSUM"))
    qpool = ctx.enter_context(tc.tile_pool(name="qps", bufs=2, space="PSUM"))

    # ======== 1. constants + d2-operand prep (the critical startup path) ====
    ident = const.tile([128, 128], F32)
    masks.make_identity(nc, ident[:])
    identb = const.tile([128, 128], BF16)
    masks.make_identity(nc, identb[:])
    onesf = const.tile([1, BL], F32)
    nc.vector.memset(onesf[:], 1.0)
    ones4f = const.tile([4, N], F32)
    nc.gpsimd.memset(ones4f[:], 1.0)
    tbias = const.tile([128, 1], F32)
    nc.vector.memset(tbias[:], THRESH)
    b512 = const.tile([128, 1], F32)
    nc.vector.memset(b512[:], float(N) / 2.0)

    # PP [128, N] f32, per-batch 32-row blocks [px, py, r2, one, qx, qy, one, r2]
    PP = const.tile([128, N], F32)
    px_src = x_d.ap()[:, :, 1:2].rearrange("b n o -> (b o) n")
    py_src = x_d.ap()[:, :, 2:3].rearrange("b n o -> (b o) n")
    pxy4 = const.tile([4, 2 * N], F32)
    nc.sync.dma_start(out=pxy4[:, 0:N], in_=px_src)
    nc.sync.dma_start(out=pxy4[:, N:2 * N], in_=py_src)
    nc.sync.dma_start(out=PP[0:128:32, :], in_=px_src)
    nc.sync.dma_start(out=PP[1:128:32, :], in_=py_src)
    sq4 = const.tile([4, 2 * N], F32)
    nc.vector.tensor_tensor(out=sq4[:], in0=pxy4[:], in1=pxy4[:], op=ALU.mult)
    r24 = const.tile([4, N], F32)
    nc.vector.tensor_tensor(out=r24[:], in0=sq4[:, 0:N], in1=sq4[:, N:2 * N], op=ALU.add)
    qq4 = const.tile([4, 2 * N], F32)
    nc.vector.tensor_scalar_mul(out=qq4[:], in0=pxy4[:], scalar1=-2.0)
    nc.sync.dma_start(out=PP[2:128:32, :], in_=r24[:])
    nc.gpsimd.dma_start(out=PP[7:128:32, :], in_=r24[:])
    nc.gpsimd.dma_start(out=PP[3:128:32, :], in_=ones4f[:])
    nc.gpsimd.dma_start(out=PP[6:128:32, :], in_=ones4f[:])
    nc.gpsimd.dma_start(out=PP[4:128:32, :], in_=qq4[:, 0:N])
    nc.sync.dma_start(out=PP[5:128:32, :], in_=qq4[:, N:2 * N])

    # 3-part bf16 split
    H1 = const.tile([128, N], BF16)
    H2 = const.tile([128, N], BF16)
    H3 = const.tile([128, N], BF16)
    nc.vector.tensor_copy(out=H1[:], in_=PP[:])
    D1 = const.tile([128, N], F32)
    nc.vector.tensor_tensor(out=D1[:], in0=PP[:], in1=H1[:], op=ALU.subtract)
    nc.vector.tensor_copy(out=H2[:], in_=D1[:])
    D2 = const.tile([128, N], F32)
    nc.vector.tensor_tensor(out=D2[:], in0=D1[:], in1=H2[:], op=ALU.subtract)
    nc.vector.tensor_copy(out=H3[:], in_=D2[:])
    HPARTS = (H1, H2, H3)

    L_all = const.tile([128, N], BF16)
    R_all = const.tile([128, N], BF16)

    def emit_lr(b, r_on_pool):
        base = 32 * b
        for g in range(6):
            nc.sync.dma_start(out=L_all[base + 4 * g:base + 4 * g + 4, :],
                              in_=HPARTS[LA[g]][base:base + 4, :])
            eng = nc.gpsimd
            eng.dma_start(out=R_all[base + 4 * g:base + 4 * g + 4, :],
                          in_=HPARTS[RA[g]][base + 4:base + 8, :])

    # ======== per-batch pipeline pieces ========
    srow = const.tile([128, BL, NT], F32)
    dmax = const.tile([128, BL, NT], F32)
    c_all = const.tile([128, 2, BL, 2], F32)
    s_fulls, dinvs, aTs = {}, {}, {}
    relu_state = [0]

    def emit_phase_a(b):
        base = 32 * b
        s_full = spool.tile([128, NT, N], BF16, tag="s")
        s_fulls[b] = s_full
        tp = (96, 0) if base == 96 else None
        for it in range(NT):
            d2t = d2pool.tile([128, N], F32, tag="d2")
            for hf in range(2):
                nc.tensor.matmul(d2t[:, hf * 512:(hf + 1) * 512],
                                 L_all[base:base + 24, it * 128:(it + 1) * 128],
                                 R_all[base:base + 24, hf * 512:(hf + 1) * 512],
                                 start=True, stop=True, tile_position=tp)
            nc.scalar.activation(out=s_full[:, it, :], in_=d2t[:], func=AF.Sign,
                                 bias=tbias[:], scale=-1.0,
                                 accum_out=srow[:, b, it:it + 1])
            nc.vector.reduce_max(out=dmax[:, b, it:it + 1], in_=d2t[:, it * 128:N],
                                 axis=mybir.AxisListType.X)

    def emit_chain(b):
        s_full = s_fulls[b]
        dinv = work.tile([128, NT], F32, tag="dinv")
        dinvs[b] = dinv
        sq = work.tile([128, NT], F32, tag="sqdeg")
        nc.scalar.activation(out=sq[:], in_=srow[:, b, :], func=AF.Sqrt,
                             bias=b512[:], scale=0.5)
        nc.vector.reciprocal(out=dinv[:], in_=sq[:])

        zf = work.tile([128, NT, 3], F32, tag="zf")
        nc.vector.tensor_tensor(out=zf[:, :, 0:2], in0=X[:, b, :, 1:3],
                                in1=dinv[:, :, None].to_broadcast((128, NT, 2)),
                                op=ALU.mult)
        nc.vector.tensor_copy(out=zf[:, :, 2:3], in_=dinv[:, :, None])
        zext = work.tile([128, NT, 6], BF16, tag="zext")
        nc.vector.tensor_copy(out=zext[:, :, 0:3], in_=zf[:])
        zlf = work.tile([128, NT, 3], F32, tag="zlf")
        nc.vector.tensor_tensor(out=zlf[:], in0=zf[:], in1=zext[:, :, 0:3],
                                op=ALU.subtract)
        nc.vector.tensor_copy(out=zext[:, :, 3:6], in_=zlf[:])
        zhalf = work.tile([128, NT, 6], BF16, tag="zhalf")
        nc.vector.tensor_scalar_mul(out=zhalf[:], in0=zext[:], scalar1=0.5)

        zred = work.tile([128, 3], F32, tag="zred")
        nc.vector.tensor_reduce(out=zred[:], in_=zf[:].rearrange("p t c -> p c t"),
                                axis=mybir.AxisListType.X, op=ALU.add)
        csh = work.tile([128, 3], F32, tag="csh")
        nc.gpsimd.partition_all_reduce(csh[:], zred[:], channels=128,
                                       reduce_op=bass_isa.ReduceOp.add)
        nc.vector.tensor_scalar_mul(out=csh[:], in0=csh[:], scalar1=0.5)

        twsb = work.tile([6, N], F32, tag="twsb")
        for hf in range(2):
            szp = szpool.tile([6, 512], F32, tag="sz")
            for jt in range(NT):
                nc.tensor.matmul(szp[:], zhalf[:, jt, :],
                                 s_full[:, jt, hf * 512:(hf + 1) * 512],
                                 start=(jt == 0), stop=(jt == NT - 1))
            if hf == 0:
                nc.scalar.copy(out=twsb[:, 0:512], in_=szp[:])
            else:
                nc.vector.tensor_copy(out=twsb[:, 512:1024], in_=szp[:])

        twnp = smpool.tile([128, NT, 6], F32, tag="sm")
        for it in range(NT):
            nc.tensor.transpose(twnp[:, it, :], twsb[:, it * 128:(it + 1) * 128],
                                ident[:6, :6])
        tw = work.tile([128, NT, 6], F32, tag="twnp_sb")
        nc.vector.tensor_copy(out=tw[:], in_=twnp[:])

        t3 = work.tile([128, NT, 3], F32, tag="t3")
        nc.vector.tensor_tensor(out=t3[:], in0=tw[:, :, 0:3], in1=tw[:, :, 3:6],
                                op=ALU.add)
        nc.vector.tensor_tensor(out=t3[:], in0=t3[:],
                                in1=csh[:, None, :].to_broadcast((128, NT, 3)),
                                op=ALU.add)
        m1 = work.tile([128, NT], F32, tag="m1")
        nc.vector.tensor_tensor(out=m1[:], in0=t3[:, :, 2], in1=dinv[:], op=ALU.mult)
        m2 = work.tile([128, NT], F32, tag="m2")
        nc.vector.tensor_tensor(out=m2[:], in0=m1[:], in1=dinv[:], op=ALU.mult)
        a3 = work.tile([128, NT, 3], F32, tag="a3")
        nc.vector.tensor_tensor(out=a3[:, :, 0:2], in0=t3[:, :, 0:2],
                                in1=m2[:, :, None].to_broadcast((128, NT, 2)),
                                op=ALU.mult)
        nc.vector.tensor_scalar_mul(out=a3[:, :, 0:2], in0=a3[:, :, 0:2], scalar1=1.0 / N)
        nc.vector.tensor_scalar_mul(out=a3[:, :, 2:3], in0=m1[:, :, None], scalar1=1.0 / N)

        a9 = work.tile([128, NT, 9], BF16, tag="a9")
        nc.vector.tensor_copy(out=a9[:, :, 0:3], in_=a3[:])
        nc.vector.tensor_copy(out=a9[:, :, 3:6], in_=a3[:])
        alf = work.tile([128, NT, 3], F32, tag="alf")
        nc.vector.tensor_tensor(out=alf[:], in0=a3[:], in1=a9[:, :, 0:3],
                                op=ALU.subtract)
        nc.vector.tensor_copy(out=a9[:, :, 6:9], in_=alf[:])

        atps = smpool.tile([9, N], BF16, tag="sm")
        for it in range(NT):
            nc.tensor.transpose(atps[:, it * 128:(it + 1) * 128], a9[:, it, :], identb[:])
        aT = work.tile([9, N], BF16, tag="aT")
        nc.vector.tensor_copy(out=aT[:], in_=atps[:])
        aTs[b] = aT

    def emit_q(b):
        aT = aTs[b]
        for mt in range(2):
            for hf in range(2):
                qps = qpool.tile([128, 512], F32, tag="q")
                nc.tensor.matmul(qps[:], wq9[:, mt * 128:(mt + 1) * 128],
                                 aT[:, hf * 512:(hf + 1) * 512], start=True, stop=True)
                rl = scratch.tile([128, 512], BF16, tag="rl")
                if relu_state[0] < 5 and hf == 0:
                    relu_state[0] += 1
                    nc.vector.tensor_scalar_max(out=rl[:], in0=qps[:], scalar1=0.0)
                    nc.vector.tensor_reduce(out=c_all[:, mt, b, hf:hf + 1], in_=rl[:],
                                            axis=mybir.AxisListType.X, op=ALU.add)
                else:
                    nc.scalar.activation(out=rl[:], in_=qps[:], func=AF.Relu,
                                         accum_out=c_all[:, mt, b, hf:hf + 1])

    # ======== 2. batch 0 front-loaded ========
    emit_lr(0, r_on_pool=False)
    emit_phase_a(0)
    for b in range(1, BL):
        emit_lr(b, r_on_pool=True)

    # ======== 3. bulk input staging (overlaps batch-0 compare) ========
    X = const.tile([128, BL, NT, FEAT], F32)
    nc.sync.dma_start(out=X[:], in_=x_d.ap().rearrange("b (t p) f -> p b t f", p=128))
    xfT = const.tile([128, NKT, BL], F32)
    xf_flat = x_d.ap().rearrange("b n f -> b (n f)")
    for bb in range(BL):
        nc.sync.dma_start(
            out=xfT[:, :, bb:bb + 1],
            in_=xf_flat[bb:bb + 1, :].rearrange("b (kt p) -> p kt b", p=128),
        )
    wm0nat = const.tile([MLP_H, NKT, 128], F32)
    nc.sync.dma_start(out=wm0nat[:], in_=wm0_d.ap().rearrange("m (kt f) -> m kt f", f=128))
    w2nat = const.tile([128, 2, HID], F32)
    nc.sync.dma_start(out=w2nat[:], in_=w2_d.ap().rearrange("(mt p) k -> p mt k", p=128))
    wfcnat = const.tile([128, 2, HID], F32)
    nc.sync.dma_start(out=wfcnat[:], in_=wfc_d.ap().rearrange("(mt p) k -> p mt k", p=128))
    wm1nat = const.tile([MLP_H, MLP_H], F32)
    nc.sync.dma_start(out=wm1nat[:], in_=wm1_d.ap())
    b2np = const.tile([128, 2], F32)
    nc.sync.dma_start(out=b2np[:], in_=b2_d.ap().rearrange("(mt p) -> p mt", p=128))
    bfcnp = const.tile([128, 2], F32)
    nc.sync.dma_start(out=bfcnp[:], in_=bfc_d.ap().rearrange("(mt p) -> p mt", p=128))
    bm0np = const.tile([MLP_H, 1], F32)
    nc.sync.dma_start(out=bm0np[:], in_=bm0_d.ap().rearrange("(p o) -> p o", o=1))
    bm1np = const.tile([MLP_H, 1], F32)
    nc.sync.dma_start(out=bm1np[:], in_=bm1_d.ap().rearrange("(p o) -> p o", o=1))
    wside = const.tile([3, HID], F32)
    nc.sync.dma_start(out=wside[0:2, :], in_=w1_d.ap().rearrange("h i -> i h"))
    nc.sync.dma_start(out=wside[2:3, :], in_=b1_d.ap().rearrange("(o h) -> o h", o=1))
    wh = const.tile([3, HID], BF16)
    nc.vector.tensor_copy(out=wh[:], in_=wside[:])
    wlf = const.tile([3, HID], F32)
    nc.vector.tensor_tensor(out=wlf[:], in0=wside[:], in1=wh[:], op=ALU.subtract)
    wl = const.tile([3, HID], BF16)
    nc.vector.tensor_copy(out=wl[:], in_=wlf[:])
    wq9 = const.tile([9, HID], BF16)
    nc.sync.dma_start(out=wq9[0:3, :], in_=wh[:])
    nc.sync.dma_start(out=wq9[3:6, :], in_=wl[:])
    nc.sync.dma_start(out=wq9[6:9, :], in_=wh[:])
    wgte = const.tile([3, 8], F32)
    nc.sync.dma_start(out=wgte[0:2, :], in_=wg_d.ap().rearrange("o i -> i o"))
    nc.sync.dma_start(out=wgte[2:3, :], in_=bg_d.ap().rearrange("(o h) -> o h", o=1))
    wpte0 = const.tile([73, 8], F32)
    nc.sync.dma_start(out=wpte0[0:64, :], in_=wp_d.ap()[:, 0:64].rearrange("o k -> k o"))
    nc.sync.dma_start(out=wpte0[64:72, :], in_=wp_d.ap()[:, 320:328].rearrange("o k -> k o"))
    nc.sync.dma_start(out=wpte0[72:73, :], in_=bp_d.ap().rearrange("(o h) -> o h", o=1))
    wpt1 = const.tile([128, 8], F32)
    nc.sync.dma_start(out=wpt1[:], in_=wp_d.ap()[:, 64:192].rearrange("o k -> k o"))
    wpt2 = const.tile([128, 8], F32)
    nc.sync.dma_start(out=wpt2[:], in_=wp_d.ap()[:, 192:320].rearrange("o k -> k o"))

    # avg-speed chain: only needs X
    vsq = const.tile([128, BL, NT, 2], F32)
    nc.vector.tensor_tensor(out=vsq[:], in0=X[:, :, :, 3:5], in1=X[:, :, :, 3:5],
                            op=ALU.mult)
    vs2 = const.tile([128, BL, NT], F32)
    nc.vector.tensor_tensor(out=vs2[:], in0=vsq[:, :, :, 0], in1=vsq[:, :, :, 1],
                            op=ALU.add)
    spd = const.tile([128, BL, NT], F32)
    nc.scalar.activation(out=spd[:], in_=vs2[:], func=AF.Sqrt)
    spr = const.tile([128, BL], F32)
    nc.vector.tensor_reduce(out=spr[:], in_=spd[:], axis=mybir.AxisListType.X,
                            op=ALU.add)
    spsum = const.tile([128, BL], F32)
    nc.gpsimd.partition_all_reduce(spsum[:], spr[:], channels=128,
                                   reduce_op=bass_isa.ReduceOp.add)

    # ======== 4. phase-0 PE work, split into pieces spread through the loop ==
    wm0T = const.tile([128, NKT, MLP_H], F32)
    wfcT = const.tile([128, 2, HID], F32)
    ct = const.tile([128, 2, HID], F32)
    bcomb = const.tile([128, 2], F32)
    wm1T = const.tile([MLP_H, MLP_H], F32)
    cat0 = const.tile([128, BL], F32)
    m1sb = const.tile([MLP_H, BL], F32)

    def emit_wm0t(half):
        for grp in range(4 * half, 4 * half + (4 if half == 0 else 3)):
            pt = qpool.tile([128, 512], F32, tag="q")
            for j in range(8):
                kt = grp * 8 + j
                nc.tensor.transpose(pt[:, j * 64:(j + 1) * 64], wm0nat[:, kt, :],
                                    ident[:MLP_H, :MLP_H])
            nc.vector.tensor_copy(
                out=wm0T[:, grp * 8:(grp + 1) * 8, :].rearrange("p a b -> p (a b)"),
                in_=pt[:])

    def emit_gcn_head():
        for kt in range(2):
            pt = qpool.tile([128, 512], F32, tag="q")
            for mt in range(2):
                nc.tensor.transpose(pt[:, mt * 128:(mt + 1) * 128],
                                    wfcnat[:, mt, kt * 128:(kt + 1) * 128], ident[:])
            nc.vector.tensor_copy(out=wfcT[:, kt, :], in_=pt[:, 0:256])
        # CT = W2^T @ Wfc^T ; bcomb = Wfc@b2 + bfc
        for it_ in range(2):
            pt = qpool.tile([128, 512], F32, tag="q")
            for kt in range(2):
                nc.tensor.matmul(pt[:, 0:256], w2nat[:, kt, it_ * 128:(it_ + 1) * 128],
                                 wfcT[:, kt, :], start=(kt == 0), stop=(kt == 1))
            nc.vector.tensor_copy(out=ct[:, it_, :], in_=pt[:, 0:256])
        for mt in range(2):
            pt = qpool.tile([128, 512], F32, tag="q")
            for kt in range(2):
                nc.tensor.matmul(pt[:, 0:1], wfcT[:, kt, mt * 128:(mt + 1) * 128],
                                 b2np[:, kt:kt + 1], start=(kt == 0), stop=(kt == 1))
            nc.scalar.activation(out=bcomb[:, mt:mt + 1], in_=pt[:, 0:1],
                                 func=AF.Identity, bias=bfcnp[:, mt:mt + 1], scale=1.0)

    def emit_mlp():
        pt = qpool.tile([128, 512], F32, tag="q")
        nc.tensor.transpose(pt[:MLP_H, :MLP_H], wm1nat[:], ident[:MLP_H, :MLP_H])
        nc.vector.tensor_copy(out=wm1T[:], in_=pt[:MLP_H, :MLP_H])
        nc.gpsimd.memset(cat0[:], 0.0)
        nc.sync.dma_start(out=cat0[72:73, :], in_=onesf[:])
        m1ps = qpool.tile([MLP_H, 512], F32, tag="q")
        for kt in range(NKT):
            nc.tensor.matmul(m1ps[:, 0:BL], wm0T[:, kt, :], xfT[:, kt, :],
                             start=(kt == 0), stop=(kt == NKT - 1))
        nc.scalar.activation(out=m1sb[:], in_=m1ps[:, 0:BL], func=AF.Relu,
                             bias=bm0np[:], scale=1.0)
        m2ps = qpool.tile([MLP_H, 512], F32, tag="q")
        nc.tensor.matmul(m2ps[:, 0:BL], wm1T[:], m1sb[:], start=True, stop=True)
        nc.scalar.activation(out=cat0[0:64, :], in_=m2ps[:, 0:BL], func=AF.Relu,
                             bias=bm1np[:], scale=1.0)

    # ======== 5. staggered pipeline with phase-0 fillers ====================
    emit_phase_a(1)
    emit_chain(0)
    emit_wm0t(0)
    emit_phase_a(2)
    emit_chain(1)
    emit_q(0)
    emit_wm0t(1)
    emit_phase_a(3)
    emit_chain(2)
    emit_q(1)
    emit_gcn_head()
    emit_chain(3)
    emit_q(2)
    emit_mlp()
    emit_q(3)

    # ======== 6. final chain ========
    cm = const.tile([128, 2, BL], F32)
    nc.vector.tensor_tensor(out=cm[:], in0=c_all[:, :, :, 0], in1=c_all[:, :, :, 1],
                            op=ALU.add)
    g2sb = const.tile([128, 2, BL], F32)
    for mt in range(2):
        gps = qpool.tile([128, 512], F32, tag="q")
        for kt in range(2):
            nc.tensor.matmul(gps[:, 0:BL], ct[:, kt, mt * 128:(mt + 1) * 128],
                             cm[:, kt, :], start=(kt == 0), stop=(kt == 1))
        nc.scalar.activation(out=g2sb[:, mt, :], in_=gps[:, 0:BL], func=AF.Identity,
                             bias=bcomb[:, mt:mt + 1], scale=1.0)

    dmr = const.tile([128, BL], F32)
    nc.vector.tensor_reduce(out=dmr[:], in_=dmax[:], axis=mybir.AxisListType.X,
                            op=ALU.max)
    dmx = const.tile([128, BL], F32)
    nc.gpsimd.partition_all_reduce(dmx[:], dmr[:], channels=128,
                                   reduce_op=bass_isa.ReduceOp.max)
    gloin = const.tile([3, BL], F32)
    nc.vector.tensor_scalar_mul(out=gloin[0:1, :], in0=spsum[0:1, :], scalar1=1.0 / N)
    dsq = const.tile([1, BL], F32)
    nc.scalar.activation(out=dsq[:], in_=dmx[0:1, :], func=AF.Sqrt)
    drc = const.tile([1, BL], F32)
    nc.vector.reciprocal(out=drc[:], in_=dsq[:])
    nc.sync.dma_start(out=gloin[1:2, :], in_=drc[:])
    nc.sync.dma_start(out=gloin[2:3, :], in_=onesf[:])

    glops = qpool.tile([8, 512], F32, tag="q")
    nc.tensor.matmul(glops[:, 0:BL], wgte[:], gloin[:], start=True, stop=True)
    nc.scalar.activation(out=cat0[64:72, :], in_=glops[:, 0:BL], func=AF.Relu)

    ops = qpool.tile([8, 512], F32, tag="q")
    nc.tensor.matmul(ops[:, 0:BL], wpte0[:], cat0[0:73, :], start=True, stop=False)
    nc.tensor.matmul(ops[:, 0:BL], wpt1[:], g2sb[:, 0, :], start=False, stop=False)
    nc.tensor.matmul(ops[:, 0:BL], wpt2[:], g2sb[:, 1, :], start=False, stop=True)
    outsb = const.tile([8, BL], F32)
    nc.vector.tensor_copy(out=outsb[:], in_=ops[:, 0:BL])
    nc.sync.dma_start(out=out_d.ap().rearrange("b o -> o b"), in_=outsb[:])

    ctx.close()


def _get_nc():
    if "nc" not in _NC_CACHE:
        _NC_CACHE["nc"] = _build()
    return _NC_CACHE["nc"]


def _prep_inputs(inputs):
    prepped = {}
    for k, v in inputs.items():
        a = np.asarray(v)
        if a.dtype != np.float32:
            a = a.astype(np.float32)
        prepped[k] = np.ascontiguousarray(a)
    return prepped


def run_sharded(inputs, **kwargs):
    """Build per-core in_maps (batch-sharded x, replicated weights) and run."""
    inputs = _prep_inputs(inputs)
    nc = _get_nc()
    x = inputs["x"]
    in_maps = []
    for c in range(NCORES):
        m = {k: v for k, v in inputs.items() if k != "x"}
        m["x"] = np.ascontiguousarray(x[c * BL:(c + 1) * BL])
        in_maps.append(m)
    res = run_bass_kernel_spmd(nc, in_maps, core_ids=list(range(NCORES)), **kwargs)
    out = np.concatenate([res.results[c]["out"] for c in range(NCORES)], axis=0)
    return out, res


def kernel(**inputs) -> np.ndarray:
    out, _ = run_sharded(inputs)
    return out

